# revision 39
# baseline (speedup 1.0000x reference)
"""Trainium2 Bass kernel for nn_ActorSpine (population-coding encoder MLP actor).

Reference computation (per sample):
  spine = sigmoid((state[:, :, None] - mean_enc) / std_enc)  # [B, 128, 10]
  a1 = relu(spine.reshape(B, 1280) @ W1.T + b1)              # [B, 2048]
  a2 = relu(a1 @ W2.T + b2)                                  # [B, 2048]
  a3 = a2 @ W3.T + b3                                        # [B, 320]
  raw = einsum('bak,ak->ba', a3.reshape(B, 32, 10), Wd[:, 0]) + bd
  out = tanh(raw)                                            # [B, 32]

Strategy: pure data parallel over 8 cores (2048 samples each).
Host-side folding:
  - decoder conv folds into W3: W3p[a, h] = sum_k Wd[a,0,k] * W3[a*10+k, h],
    b3p[a] = sum_k Wd[a,0,k]*b3[a*10+k] + bd[a]  -> final layer is [32, 2048]
  - encoder contraction index permuted j' = k*128 + d so spine k-tiles are
    plain per-partition sigmoid activations of stateT; W1 columns permuted to
    match.
Device: all three layers run in fp8 e4m3 with perf_mode=DoubleRow (256-row
contraction per matmul, ~1.8x the fp16 PE rate). Scales (all powers of 2,
rel err ~1.1e-2 vs 2e-2 gate):
  spine fp8 as-is (values in [0,1]); W1 scaled by 64 so the L1 relu needs
  no rescale (bias only) and h1 = 64*a1 fits fp8 (max ~120 < 240); W2
  scaled by 8192, L2 relu on ScalarE applies scale 2^-13 + bias 64*b2 ->
  fp8 h2 = 64*a2; W3 scaled by 8192, layer-3 accumulates straight into a
  [32, NT] psum bank (DoubleRow forbids dst-partition offsets, so no
  col-packing / selection-matmul reduce), tanh applies the 2^-19 rescale.
All fp8 weights are SBUF-resident (W2 loads once, not per sweep).
Per-batch-chunk sweeps interleave sigmoid -> L1 -> L2 so the ScalarE ramp
overlaps PE; a k-striped cohort over all PSUM banks turns the initial
sigmoid ramp into real L1 work; layer 3 runs as col-packed quads
(tile_position) with a selection-matmul cross-partition reduce; PSUM banks
rotate manually with DVE pre-zeroing and no-start accumulation groups.
"""

import numpy as np
import ml_dtypes

import concourse.mybir as mybir
import concourse.tile as tile
from concourse import bacc
from concourse.bass_utils import run_bass_kernel_spmd

# Problem dims (hardcoded per harness contract)
B = 16384
D = 128
ENC_K = 10
ACT_DIM = 32
DEC_K = 10
H0 = 2048
H1 = 2048
NCORES = 8
BL = B // NCORES  # 2048 samples per core
NT = 512          # moving-dim tile (one PSUM bank of fp32)
NSUB = BL // NT   # 4
M1 = H0 // 128    # 16 m-tiles for layer 1
K1 = ENC_K        # 10 k-tiles for layer 1 (permuted encoder)
K1P = K1 // 2     # 5 fp8 DoubleRow k-pairs
M2 = H1 // 128    # 16
K2 = H0 // 128    # 16
K2P = K2 // 2     # 8
K3 = H1 // 128    # 16

F8 = mybir.dt.float8e4
F16 = mybir.dt.float16
F32 = mybir.dt.float32
DR = mybir.MatmulPerfMode.DoubleRow

S_W1 = 64.0      # W1 fp8 scale == h1 scale, so L1 relu is plain add+max
S_W2 = 8192.0    # W2 fp8 scale
S_L2 = 2.0 ** -13  # ScalarE L2 relu scale: psum*2^-13 + 64*b2 -> h2 = 64*a2
S_W3 = 8192.0    # W3 fp8 scale

_cached = {}


def _build_program():
    if "nc" in _cached:
        return _cached["nc"]

    nc = bacc.Bacc("TRN2", target_bir_lowering=False, debug=False,
                   num_devices=NCORES)

    BF16 = mybir.dt.bfloat16
    stateT = nc.dram_tensor("stateT", [D, BL], BF16, kind="ExternalInput")
    w1t = nc.dram_tensor("w1t", [M1, 128, K1, 128], F8, kind="ExternalInput")
    w2t = nc.dram_tensor("w2t", [M2, 128, K2, 128], F8, kind="ExternalInput")
    w3t = nc.dram_tensor("w3t", [128, K3, ACT_DIM], F8, kind="ExternalInput")
    # scalars layout (per partition p): [0:10] enc_scale, [10:20] enc_bias,
    # [20:36] 64*b1, [36:52] b2, [52] b3p (partitions 0..31)
    scal = nc.dram_tensor("scal", [128, 53], F32, kind="ExternalInput")
    out = nc.dram_tensor("out", [ACT_DIM, BL], F32, kind="ExternalOutput")

    with tile.TileContext(nc) as tc:
        with (
            tc.tile_pool(name="consts", bufs=1) as consts,
            tc.tile_pool(name="acts", bufs=1) as acts,
            tc.tile_pool(name="h2p", bufs=10) as h2p,
            tc.tile_pool(name="w1p", bufs=1) as w1p,
            tc.tile_pool(name="w2p", bufs=1) as w2p,
            tc.tile_pool(name="outp", bufs=2) as outp,
            tc.tile_pool(name="psum", bufs=1, space="PSUM") as psum_pool,
        ):
            # scal goes first: its descriptors must not queue behind the
            # bulk transfers (the first sigmoid waits on it); then state
            # chunk 0, which gates the same sigmoid but transfers fast
            sc = consts.tile([128, 53], F32)
            nc.sync.dma_start(out=sc, in_=scal[:, :])
            st = acts.tile([D, BL], BF16, tag="state")
            nc.sync.dma_start(out=st[:, 0:NT], in_=stateT[:, 0:NT])

            # Persistent PSUM accumulators, rotated manually. Banks are
            # zeroed by DVE several groups before reuse, and matmul groups
            # run WITHOUT start=True: the group-start bank-clear blocks the
            # LDWEIGHTS pull-ahead and costs ~100ns per group.
            NPS = 7
            ps_tiles = [psum_pool.tile([128, NT], F32, tag=f"ps{i}",
                                       name=f"ps{i}")
                        for i in range(NPS)]
            ps_idx = [0]

            def next_ps():
                t = ps_tiles[ps_idx[0] % NPS]
                ps_idx[0] += 1
                return t

            # layer-3 accumulates straight into a [32, NT] bank (DoubleRow
            # forbids nonzero dst-partition offsets, so no col-packing)
            psr = psum_pool.tile([ACT_DIM, NT], F32, tag="psr", name="psr")

            # ---- PE warmup: dummy matmuls on a zeroed tile so the HAM
            # clock-gate opens during the initial state/weight DMA window.
            # Also zeroes all accumulator banks for the no-start scheme.
            # wz is zeroed by a fast DVE memset (not the ScalarE table-warm
            # ops, whose ACT_TABLE_LOAD would delay the first matmul ~2us);
            # the table-warm activations write a separate scratch tile.
            wz = consts.tile([128, NT], F16, tag="warmzero")
            nc.vector.memset(wz, 0.0)
            twz = consts.tile([1, 2], F16, tag="tablewarm")
            nc.scalar.activation(twz[0:1, 0:1], wz[0:1, 0:1],
                                 mybir.ActivationFunctionType.Sigmoid)
            nc.scalar.activation(twz[0:1, 1:2], wz[0:1, 1:2],
                                 mybir.ActivationFunctionType.Tanh)
            NWARM = 8
            for w in range(NWARM):
                nc.tensor.matmul(
                    psr, wz[:, :ACT_DIM], wz,
                    start=(w == 0), stop=(w == NWARM - 1),
                    skip_group_check=True)
            for t in ps_tiles + [psr]:
                nc.vector.memset(t, 0.0)

            # W1 resident in SBUF (one block per m-tile, loaded once);
            # the cohort blocks (first NPS) load before the remaining state
            # chunks so sweep-0's k-waves are not DMA-gated
            w1sb = []
            for m in range(M1):
                w1m = w1p.tile([128, K1, 128], F8, tag=f"w1_{m}",
                               name=f"w1_{m}")
                w1sb.append(w1m)
            # all W1 tiles before the remaining state chunks: the DMA
            # pipeline delivers ~1 tile / 0.8us and sweep-0's L1 m-loop
            # consumes them at that same rate, while state chunks 1-3 are
            # not needed until sweep 1 (~60us in)
            for m in range(M1):
                nc.sync.dma_start(out=w1sb[m], in_=w1t[m])
            for n in range(1, NSUB):
                nc.sync.dma_start(out=st[:, n * NT:(n + 1) * NT],
                                  in_=stateT[:, n * NT:(n + 1) * NT])
            # W2 is fp8-resident too (4 MB total); first needed ~20us in,
            # by which time the startup-critical transfers are done
            w2sb = []
            for m in range(M2):
                w2m = w2p.tile([128, K2, 128], F8, tag=f"w2_{m}",
                               name=f"w2_{m}")
                w2sb.append(w2m)
                nc.sync.dma_start(out=w2m, in_=w2t[m])
            # w3 is not needed until layer 3 of sweep 0 (~30us in)
            w3sb = consts.tile([128, K3, ACT_DIM], F8, tag="w3")
            nc.sync.dma_start(out=w3sb, in_=w3t[:, :, :])

            # fp8 activations, pair-packed for DoubleRow: tile [:, i, :] is
            # feature block 2q+i
            spine = [acts.tile([128, 2, BL], F8, tag=f"spine{q}",
                               name=f"spine{q}")
                     for q in range(K1P)]
            h1 = [acts.tile([128, 2, BL], F8, tag=f"h1_{q}", name=f"h1_{q}")
                  for q in range(K2P)]

            def emit_sigmoid_pair(n, q):
                ns = slice(n * NT, (n + 1) * NT)
                for k in (2 * q, 2 * q + 1):
                    nc.scalar.activation(
                        spine[k // 2][:, k % 2:k % 2 + 1, ns], st[:, ns],
                        mybir.ActivationFunctionType.Sigmoid,
                        bias=sc[:, 10 + k:11 + k], scale=sc[:, k:k + 1])

            # layer-3: 8 DoubleRow matmuls accumulating straight into the
            # [32, NT] psr bank (psum holds 64*8192*raw); tanh applies the
            # 2^-19 rescale and the folded bias.
            def emit_l3(n, h2pairs):
                for q in range(K2P):
                    nc.tensor.matmul(
                        psr, w3sb[:, 2 * q:2 * q + 2, :], h2pairs[q],
                        start=False, stop=False, skip_group_check=True,
                        perf_mode=DR)
                ot = outp.tile([ACT_DIM, NT], F32, tag="ot",
                               name=f"ot_{n}")
                nc.scalar.activation(
                    ot, psr, mybir.ActivationFunctionType.Tanh,
                    bias=sc[:ACT_DIM, 52:53], scale=2.0 ** -19)
                nc.vector.memset(psr, 0.0)
                nc.sync.dma_start(out=out[:, n * NT:(n + 1) * NT],
                                  in_=ot)

            # ---- fully interleaved per-column-chunk sweeps:
            # sigmoid(n) -> L1 m-sweep(n) -> L2 m-sweep(n) [+ lagged L3/tanh]
            for n in range(NSUB):
                ns = slice(n * NT, (n + 1) * NT)
                if n == 0:
                    for q in range(K1P):
                        emit_sigmoid_pair(0, q)

                m_start = 0
                if n == 0:
                    # k-striped cohort over all 5 banks: each k-wave only
                    # needs one freshly produced spine pair, so real L1 work
                    # runs during the ScalarE sigmoid ramp.
                    m_start = NPS
                    cohort = [next_ps() for _ in range(NPS)]
                    for q in range(K1P):
                        for m in range(NPS):
                            nc.tensor.matmul(
                                cohort[m], w1sb[m][:, 2 * q:2 * q + 2, :],
                                spine[q][:, :, ns],
                                start=False, stop=False,
                                skip_group_check=True, perf_mode=DR)
                    for m in range(NPS):
                        nc.scalar.activation(
                            h1[m // 2][:, m % 2:m % 2 + 1, ns], cohort[m],
                            mybir.ActivationFunctionType.Relu,
                            bias=sc[:, 20 + m:21 + m])
                        nc.vector.memset(cohort[m], 0.0)

                for m in range(m_start, M1):
                    ps = next_ps()
                    for q in range(K1P):
                        nc.tensor.matmul(
                            ps, w1sb[m][:, 2 * q:2 * q + 2, :],
                            spine[q][:, :, ns],
                            start=False, stop=False, skip_group_check=True,
                            perf_mode=DR)
                    nc.scalar.activation(
                        h1[m // 2][:, m % 2:m % 2 + 1, ns], ps,
                        mybir.ActivationFunctionType.Relu,
                        bias=sc[:, 20 + m:21 + m])
                    nc.vector.memset(ps, 0.0)

                h2pairs = []
                for m in range(M2):
                    ps = next_ps()
                    for q in range(K2P):
                        nc.tensor.matmul(
                            ps, w2sb[m][:, 2 * q:2 * q + 2, :],
                            h1[q][:, :, ns],
                            start=False, stop=False, skip_group_check=True,
                            perf_mode=DR)
                    if m % 2 == 0:
                        h2pairs.append(h2p.tile([128, 2, NT], F8, tag="h2",
                                                name=f"h2_{n}_{m // 2}"))
                    nc.scalar.activation(
                        h2pairs[m // 2][:, m % 2:m % 2 + 1, :], ps,
                        mybir.ActivationFunctionType.Relu,
                        bias=sc[:, 36 + m:37 + m], scale=S_L2)
                    nc.vector.memset(ps, 0.0)
                    # next sweep's sigmoid pairs ride along inside the L2
                    # m-loop (ScalarE has slack here); a single burst at
                    # sweep end would delay the L1 relus of sweep n+1 and
                    # stall its bank rotation
                    if n + 1 < NSUB and m < K1P:
                        emit_sigmoid_pair(n + 1, m)
                # flush the sweep's layer 3 as one block: a single PE seam
                # (all h2 pairs are ready by the end of the L2 m-loop)
                emit_l3(n, h2pairs)

    nc.compile()
    _cached["nc"] = nc
    return nc


def _q8(x, scale):
    # TRN fp8e4 clips at +-240 (not OCP's 448); ml_dtypes float8_e4m3
    # matches the TRN format exactly for finite values
    return np.clip(x * scale, -240.0, 240.0).astype(ml_dtypes.float8_e4m3)


def _prep_inputs(state, mean_enc, std_enc, W1, b1, W2, b2, W3, b3, Wd, bd):
    f32 = np.float32
    state = np.asarray(state, f32)
    mean_enc = np.asarray(mean_enc, f32)
    std_enc = np.asarray(std_enc, f32)
    W1 = np.asarray(W1, f32)
    b1 = np.asarray(b1, f32)
    W2 = np.asarray(W2, f32)
    b2 = np.asarray(b2, f32)
    W3 = np.asarray(W3, f32)
    b3 = np.asarray(b3, f32)
    Wd = np.asarray(Wd, f32)
    bd = np.asarray(bd, f32)

    # Fold decoder grouped conv into layer 3
    wd = Wd[:, 0, :]                                   # [32, 10]
    W3p = np.einsum("ak,akh->ah", wd, W3.reshape(ACT_DIM, DEC_K, H1))
    b3p = (b3.reshape(ACT_DIM, DEC_K) * wd).sum(1) + bd  # [32]

    # Permute encoder contraction: j' = k*128 + d
    W1p = W1.reshape(H0, D, ENC_K).transpose(0, 2, 1).reshape(H0, D * ENC_K)

    # Pre-tiled weight layouts: [m, p, k, j] = lhsT tile stack
    w1t = np.ascontiguousarray(
        _q8(W1p, S_W1).reshape(M1, 128, K1, 128).transpose(0, 3, 2, 1))
    w2t = np.ascontiguousarray(
        _q8(W2, S_W2).reshape(M2, 128, K2, 128).transpose(0, 3, 2, 1))
    w3t = np.ascontiguousarray(
        _q8(W3p, S_W3).reshape(ACT_DIM, K3, 128).transpose(2, 1, 0))

    scal = np.zeros((128, 53), f32)
    scal[:, 0:10] = 1.0 / std_enc[0]                   # enc scale [128, 10]
    scal[:, 10:20] = -mean_enc[0] / std_enc[0]         # enc bias
    scal[:, 20:36] = S_W1 * b1.reshape(M1, 128).T
    scal[:, 36:52] = S_W1 * b2.reshape(M2, 128).T
    scal[:ACT_DIM, 52] = b3p

    in_maps = []
    for c in range(NCORES):
        shard = np.ascontiguousarray(
            state[c * BL:(c + 1) * BL].T.astype(ml_dtypes.bfloat16))
        in_maps.append({
            "stateT": shard, "w1t": w1t, "w2t": w2t, "w3t": w3t,
            "scal": scal,
        })
    return in_maps


def kernel(**inputs):
    nc = _build_program()
    in_maps = _prep_inputs(**inputs)
    res = run_bass_kernel_spmd(nc, in_maps, core_ids=list(range(NCORES)))
    out = np.concatenate(
        [res.results[c]["out"].T for c in range(NCORES)], axis=0)
    return np.ascontiguousarray(out.astype(np.float32))


if __name__ == "__main__":
    rng = np.random.default_rng(0)
    state = rng.standard_normal((B, D), dtype=np.float32)
    mean = np.broadcast_to(
        np.linspace(-3, 3, ENC_K, dtype=np.float32), (1, D, ENC_K)).copy()
    std = np.full((1, D, ENC_K), 0.3872983346207417, np.float32)

    def lin(fan_in, fan_out):
        bound = 1 / np.sqrt(fan_in)
        return (rng.uniform(-bound, bound, (fan_out, fan_in)).astype(np.float32),
                rng.uniform(-bound, bound, fan_out).astype(np.float32))

    W1, b1 = lin(D * ENC_K, H0)
    W2, b2 = lin(H0, H1)
    W3, b3 = lin(H1, ACT_DIM * DEC_K)
    Wd = rng.uniform(-0.3, 0.3, (ACT_DIM, 1, DEC_K)).astype(np.float32)
    bd = rng.uniform(-0.3, 0.3, ACT_DIM).astype(np.float32)

    outp = kernel(state=state, mean_enc=mean, std_enc=std, W1=W1, b1=b1,
                  W2=W2, b2=b2, W3=W3, b3=b3, Wd=Wd, bd=bd)

    # numpy reference
    spine = 1 / (1 + np.exp(-(state[:, :, None] - mean) / std))
    a = np.maximum(spine.reshape(B, -1) @ W1.T + b1, 0)
    a = np.maximum(a @ W2.T + b2, 0)
    a = a @ W3.T + b3
    raw = np.einsum("bak,ak->ba", a.reshape(B, ACT_DIM, DEC_K), Wd[:, 0]) + bd
    ref = np.tanh(raw)
    rel = np.linalg.norm(outp - ref) / np.linalg.norm(ref)
    print("rel err:", rel, "max abs diff:", np.abs(outp - ref).max())
